# revision 40
# baseline (speedup 1.0000x reference)
"""Trainium2 Bass kernel for relative-position multi-head attention
(Transformer-XL style), sharded over 8 NeuronCores by head (2 heads/core)
with row-parallel output projection (partial sums reduced on host).

S^T-direct scheme: scores are computed transposed (m on partitions,
l free) so the PV matmul consumes P^T tiles straight out of a single
fused exp-evacuation — no transposes of P at all.

Per core c (d-slice = rows 128c..128c+128 of the projection space):
  qT = Wq[ds] @ Q.T  (128, L) [+ (bq+u)/8 rows 0:64, (bq+v)/8 rows 64:128]
  kT = Wk[ds] @ K.T + bk      (64, L) per head
  v  = V @ Wv[ds].T           (L, 128)  [bv folded on host]
  per head h, l-chunk lc (512), m-tile mt (128):
    psA  = kth_mt^T @ qu_lc             (AC^T, start=True)
    psA += I^T @ bdT[:, mt*512:+512]    (BD^T add via ident matmul)
    pT   = exp(psA)                     (ACT, PSUM->SBUF bf16)
    po  += vmt[mt]^T @ pT               (PV accumulate; row 64 = Z)
  at = po[0:64] / Z;  O_part = Wo[:, ds]^T @ at  -> (1024, L) bf16
Host: out = (sum_c O_part).T + bo + bv @ Wo.T

BD^T tiles are built per l-tile by a banded matmul against
F = flip(rel_emb) (width 2176) evacuated to SBUF bf16, then a
diagonal-shift DMA (row p of the band starts at element 127-p) into
bds[l_p, m], then an XBAR-transpose DMA whose dest AP scatters
transposed row m to partition m%128, col (m//128)*512 + lt4*128,
building bdT [128, 16*512] with contiguous m-tiles for the whole
l-chunk.  BD production for iteration i+1 is interleaved into
iteration i's mt-loop.

_build_module(reps=K) repeats the whole computation K times inside one
NEFF (used by bench_reps.py to measure device time above axon dispatch
noise); the graded kernel() uses reps=1.
"""

import math
import numpy as np
import ml_dtypes

import concourse.bass as bass
import concourse.bacc as bacc
import concourse.mybir as mybir
import concourse.tile as tile
from concourse.bass_utils import run_bass_kernel_spmd
from contextlib import ExitStack

BF16 = mybir.dt.bfloat16
F32 = mybir.dt.float32
AF = mybir.ActivationFunctionType

L = 2048          # sequence length
D = 1024          # model dim
DK = 64           # head dim
NH = 16           # total heads
NCORES = 8
DH = 128          # per-core projection slice (2 heads * 64)
LT = 128          # l-tile rows
NLT = L // LT     # 16 l-tiles
LC = 512          # l-chunk for attention/PV/Wo
NLC = L // LC     # 4
BW = 2176         # band width per l-tile (2175 rounded up to 17*128)
NMT = 16          # m-tiles per head


def _build_rep(nc, tc, ctx, dram, rep):
    """Emit one full forward pass; rep suffixes keep names unique."""
    R = f"r{rep}"
    (d_qt, d_kt, d_vt, d_wqt, d_wkt, d_wvt, d_wot, d_ft,
     d_ub, d_vb, d_kb, d_identb, d_out) = dram

    const = ctx.enter_context(tc.tile_pool(name=f"const{R}", bufs=1))
    persist = ctx.enter_context(tc.tile_pool(name=f"persist{R}", bufs=1))

    # ---- persistent SBUF loads ----
    # all consts go on the gpsimd queue (in first-use order) so the
    # sync/scalar queues start streaming qx/kx immediately.
    # F on partitions 64:128 so band matmuls share base partition with lhv
    wqt_s = const.tile([128, 8 * DH], BF16)
    wkt_s = const.tile([128, 8 * DH], BF16)
    wvt_s = const.tile([128, 8 * DH], BF16)
    ft_s = const.tile([128, 4096], BF16)
    identb = const.tile([128, 128], BF16)
    wot_s = const.tile([DH, D], BF16)
    for dd, ss in ((d_wqt, wqt_s), (d_wkt, wkt_s)):
        nc.gpsimd.dma_start(
            ss[:].rearrange("p (a d) -> p a d", a=8),
            dd[:].rearrange("(a p) d -> p a d", p=128))
    nc.gpsimd.dma_start(identb[:], d_identb[:])
    nc.gpsimd.dma_start(ft_s[DK:128, :], d_ft[:])
    nc.gpsimd.dma_start(
        wvt_s[:].rearrange("p (a d) -> p a d", a=8),
        d_wvt[:].rearrange("(a p) d -> p a d", p=128))
    nc.gpsimd.dma_start(wot_s[:], d_wot[:])
    ub_s = const.tile([DH, 1], F32)
    vb_s = const.tile([DH, 1], F32)
    kb_s = const.tile([DH, 1], F32)
    nc.gpsimd.dma_start(ub_s[:], d_ub[:])
    nc.gpsimd.dma_start(vb_s[:], d_vb[:])
    nc.gpsimd.dma_start(kb_s[:], d_kb[:])

    # per-head projection outputs
    quv = [persist.tile([128, L], BF16, tag=f"quv{h}", name=f"quv{h}{R}")
           for h in range(2)]
    kth = [persist.tile([DK, L], BF16, tag=f"kth{h}", name=f"kth{h}{R}")
           for h in range(2)]
    # v tiles per (head, m-tile): (128, 65) with ones in col 64
    vmt = [[persist.tile([128, DK + 1], BF16, tag=f"v{h}_{mt}",
                         name=f"v{h}_{mt}{R}")
            for mt in range(NLT)] for h in range(2)]

    # ---- attention + output ----
    # iteration order: (lc, h); BD^T for iteration i+1 is produced during
    # iteration i's mt-loop.
    iters = [(lc, h) for lc in range(NLC) for h in range(2)]

    with tc.tile_pool(name=f"bd{R}", bufs=2) as bdpool, \
         tc.tile_pool(name=f"bdsd{R}", bufs=2) as bdsdp, \
         tc.tile_pool(name=f"bandp{R}", bufs=2, space="PSUM") as bandp, \
         tc.tile_pool(name=f"bands{R}", bufs=2) as bandsb, \
         tc.tile_pool(name=f"at{R}", bufs=2) as atp, \
         tc.tile_pool(name=f"oev{R}", bufs=2) as oev:

        bdT_tiles = {}

        def get_bdT(it):
            if it not in bdT_tiles:
                bdT_tiles[it] = bdpool.tile([128, NMT * LC], BF16,
                                            tag="bdT", name=f"bdT{R}_{it}")
            return bdT_tiles[it]

        def make_band(it, lt4, bdT):
            """band matmul + evac + diag-shift DMA + XBAR transpose into
            bdT cols [mt*512 + lt4*128 : +128] for all 16 mt."""
            lc, h = iters[it]
            ltg = lc * 4 + lt4
            l0 = ltg * LT
            b0 = 1920 - l0
            lhv = quv[h][DK:128, l0:l0 + LT]
            bsb = bandsb.tile([LT, BW], BF16, tag="bands")
            for q4 in range(4):
                bp = bandp.tile([LT, 512], F32, tag="bp")
                nc.tensor.matmul(bp[:], lhv,
                                 ft_s[DK:128, b0 + q4 * 512:
                                      b0 + (q4 + 1) * 512],
                                 start=True, stop=True)
                if q4 == 1:
                    nc.scalar.activation(bsb[:, bass.ts(q4, 512)], bp[:],
                                         AF.Copy)
                else:
                    nc.vector.tensor_copy(bsb[:, bass.ts(q4, 512)], bp[:])
            bt = bandp.tile([LT, 128], F32, tag="bp", name=f"bt{R}_{it}_{lt4}")
            nc.tensor.matmul(bt[:], lhv, ft_s[DK:128, b0 + 2048:b0 + BW],
                             start=True, stop=True)
            nc.vector.tensor_copy(bsb[:, 2048:BW], bt[:])
            # diagonal shift (SWDGE on the gpsimd queue): row p of the band
            # starts at element 127-p, giving bds[p,j] = bsb[p, 127-p+j]
            bds = bdsdp.tile([LT, L], BF16, tag="bds")
            diag = bass.AP(bsb[:].tensor, 127, [[BW - 1, LT], [1, L]])
            nc.gpsimd.dma_start(bds[:], diag)
            # XBAR transpose: transposed row j lands at partition j%128,
            # col (j//128)*512 + lt4*128, so bdT holds BD^T with contiguous
            # m-tiles of the whole l-chunk
            outap = bass.AP(bdT[:].tensor, lt4 * LT,
                            [[NMT * LC, 128], [LC, NMT], [1, LT]])
            nc.sync.dma_start_transpose(outap, bds[:])

        # ---- q/k projections (q first; the BD^T prologue for iteration 0
        # is emitted between them so its band chain overlaps k-proj) ----
        with tc.tile_pool(name=f"xin{R}", bufs=1) as xin, \
             tc.tile_pool(name=f"prjp{R}", bufs=2, space="PSUM") as prjp:
            qxs, kxs = [], []
            for cc in range(8):
                qx = xin.tile([128, L], BF16, tag=f"qx{cc}", name=f"qx{cc}{R}")
                eng = nc.sync if cc % 2 == 0 else nc.scalar
                eng.dma_start(qx[:], d_qt[cc * 128:(cc + 1) * 128, :])
                qxs.append(qx)
            for cc in range(8):
                kx = xin.tile([128, L], BF16, tag=f"kx{cc}", name=f"kx{cc}{R}")
                eng = nc.sync if cc % 2 == 1 else nc.scalar
                eng.dma_start(kx[:], d_kt[cc * 128:(cc + 1) * 128, :])
                kxs.append(kx)
            for lc4 in range(4):
                lsl = bass.ts(lc4, 512)
                qp = prjp.tile([128, 512], F32, tag="qp")
                for cc in range(8):
                    nc.tensor.matmul(qp[:], wqt_s[:, bass.ts(cc, DH)],
                                     qxs[cc][:, lsl],
                                     start=(cc == 0), stop=(cc == 7))
                for h in range(2):
                    hsl = slice(h * DK, (h + 1) * DK)
                    nc.scalar.activation(quv[h][0:DK, lsl], qp[hsl, :],
                                         AF.Identity, bias=ub_s[hsl, :],
                                         scale=0.125)
                    nc.scalar.activation(quv[h][DK:128, lsl], qp[hsl, :],
                                         AF.Identity, bias=vb_s[hsl, :],
                                         scale=0.125)
            # prologue bands for iteration 0 (band chain overlaps k-proj)
            bdT_cur = get_bdT(0)
            for lt4 in range(4):
                make_band(0, lt4, bdT_cur)
            for lc4 in range(4):
                lsl = bass.ts(lc4, 512)
                kp = prjp.tile([128, 512], F32, tag="kp")
                for cc in range(8):
                    nc.tensor.matmul(kp[:], wkt_s[:, bass.ts(cc, DH)],
                                     kxs[cc][:, lsl],
                                     start=(cc == 0), stop=(cc == 7))
                for h in range(2):
                    hsl = slice(h * DK, (h + 1) * DK)
                    nc.scalar.activation(kth[h][:, lsl], kp[hsl, :],
                                         AF.Identity, bias=kb_s[hsl, :])

        with tc.tile_pool(name=f"pa{R}", bufs=2, space="PSUM") as pa, \
             tc.tile_pool(name=f"pvp{R}", bufs=2, space="PSUM") as pvp, \
             tc.tile_pool(name=f"pt{R}", bufs=4) as ptp:

            # ---- v projection (vT layout like q/k, then DMA-transpose) ----
            vt_s = persist.tile([DH, L], BF16, tag="vt_s", name=f"vt_s{R}")
            with tc.tile_pool(name=f"vin{R}", bufs=1) as vin:
                vchunks = []
                for cc in range(8):
                    vx = vin.tile([128, L], BF16, tag=f"vx{cc}")
                    eng = nc.sync if cc % 2 == 0 else nc.scalar
                    eng.dma_start(vx[:], d_vt[cc * 128:(cc + 1) * 128, :])
                    vchunks.append(vx)
                for lc4 in range(4):
                    lsl = bass.ts(lc4, 512)
                    vtp = pa.tile([128, 512], F32, tag="a",
                                  name=f"vtp{R}_{lc4}")
                    for cc in range(8):
                        nc.tensor.matmul(vtp[:], wvt_s[:, bass.ts(cc, DH)],
                                         vchunks[cc][:, lsl],
                                         start=(cc == 0), stop=(cc == 7))
                    nc.scalar.activation(vt_s[:, lsl], vtp[:], AF.Copy)
                for h in range(2):
                    for mt in range(NLT):
                        nc.sync.dma_start_transpose(
                            vmt[h][mt][:, 0:DK],
                            vt_s[h * DK:(h + 1) * DK, bass.ts(mt, 128)])
                        nc.gpsimd.memset(vmt[h][mt][:, DK:DK + 1], 1.0)

            # rz: row 0 = 1/Z (rewritten per iter), row 1 = zeros (once)
            rz = persist.tile([2, LC], F32, tag="rz", name=f"rz{R}")
            nc.gpsimd.memset(rz[:], 0.0)

            pending_norm = [None]
            for it, (lc, h) in enumerate(iters):
                lsl = bass.ts(lc, LC)
                qu = quv[h][0:DK, lsl]
                po = pvp.tile([DK + 1, LC], F32, tag="po")
                if h == 0:
                    at_s = atp.tile([DH, LC], BF16, tag="at")
                at_cur = at_s
                prev_pv = None
                for mp in range(NMT // 2):
                    if mp < 4 and it + 1 < len(iters):
                        make_band(it + 1, mp, get_bdT(it + 1))
                    psA = pa.tile([128, 2 * LC], F32, tag="a")
                    for sub in range(2):
                        mt = 2 * mp + sub
                        msl = bass.ts(mt, 128)
                        csl = bass.ts(sub, LC)
                        nc.tensor.matmul(psA[:, csl], kth[h][:, msl], qu,
                                         start=True, stop=False)
                    for sub in range(2):
                        mt = 2 * mp + sub
                        csl = bass.ts(sub, LC)
                        nc.tensor.matmul(psA[:, csl], identb[:],
                                         bdT_cur[:, bass.ts(mt, LC)],
                                         start=False, stop=True)
                    if mp == 5 and pending_norm[0] is not None:
                        pending_norm[0]()
                        pending_norm[0] = None
                    if prev_pv is not None:
                        prev_pv()
                    pt = ptp.tile([128, 2 * LC], BF16, tag="pt")
                    nc.scalar.activation(pt[:], psA[:], AF.Exp)
                    def pv_pair(mp=mp, p=pt):
                        for sub in range(2):
                            mt = 2 * mp + sub
                            nc.tensor.matmul(po[:], vmt[h][mt][:],
                                             p[:, bass.ts(sub, LC)],
                                             start=(mt == 0),
                                             stop=(mt == NMT - 1))
                    prev_pv = pv_pair
                prev_pv()
                bdT_tiles.pop(it, None)
                bdT_cur = bdT_tiles.get(it + 1)

                def norm_and_wo(po=po, at_s=at_cur, h=h, lc=lc, lsl=lsl):
                    # normalize: recipZ broadcast to 64 partitions via a
                    # stride-0 source-AP DMA on the sync queue
                    nc.vector.reciprocal(rz[0:1, :], po[DK:DK + 1, :])
                    bcs = oev.tile([DK, LC], F32, tag="bcs")
                    bsrc = bass.AP(rz[:].tensor, 0,
                                   [[1, 1], [0, DK], [1, LC]])
                    nc.sync.dma_start(bcs[:], bsrc)
                    nc.vector.tensor_tensor(at_s[h * DK:(h + 1) * DK, :],
                                            po[0:DK, :], bcs[:],
                                            mybir.AluOpType.mult)
                    if h == 1:
                        # Wo: 8 e-tiles
                        for et in range(8):
                            wp = bandp.tile([128, LC], F32, tag="bp",
                                            name=f"wp{R}_{lc}_{et}")
                            nc.tensor.matmul(wp[:],
                                             wot_s[:, bass.ts(et, 128)],
                                             at_s[:], start=True, stop=True)
                            osb = oev.tile([128, LC], BF16, tag="osb")
                            nc.scalar.activation(osb[:], wp[:], AF.Copy)
                            nc.gpsimd.dma_start(
                                d_out[et * 128:(et + 1) * 128, lsl], osb[:])
                pending_norm[0] = norm_and_wo
            pending_norm[0]()


def _build_module(reps=1):
    nc = bacc.Bacc("TRN2", target_bir_lowering=False, debug=False,
                   enable_asserts=False, num_devices=NCORES)

    # ---- DRAM I/O ----
    dram = (
        nc.dram_tensor("qt", (D, L), BF16, kind="ExternalInput"),
        nc.dram_tensor("kt", (D, L), BF16, kind="ExternalInput"),
        nc.dram_tensor("vt", (D, L), BF16, kind="ExternalInput"),
        nc.dram_tensor("wqt", (D, DH), BF16, kind="ExternalInput"),
        nc.dram_tensor("wkt", (D, DH), BF16, kind="ExternalInput"),
        nc.dram_tensor("wvt", (D, DH), BF16, kind="ExternalInput"),
        nc.dram_tensor("wot", (DH, D), BF16, kind="ExternalInput"),
        nc.dram_tensor("ft", (DK, 4096), BF16, kind="ExternalInput"),
        nc.dram_tensor("ubias", (DH, 1), F32, kind="ExternalInput"),
        nc.dram_tensor("vbias", (DH, 1), F32, kind="ExternalInput"),
        nc.dram_tensor("kbias", (DH, 1), F32, kind="ExternalInput"),
        nc.dram_tensor("identb", (128, 128), BF16, kind="ExternalInput"),
        nc.dram_tensor("opart", (D, L), BF16, kind="ExternalOutput"),
    )

    with tile.TileContext(nc) as tc:
        for rep in range(reps):
            with ExitStack() as ctx:
                _build_rep(nc, tc, ctx, dram, rep)
    nc.compile()
    return nc


_MODULE_CACHE = {}


def _get_module(reps=1):
    if reps not in _MODULE_CACHE:
        _MODULE_CACHE[reps] = _build_module(reps)
    return _MODULE_CACHE[reps]


def make_in_maps(inputs):
    Q = np.asarray(inputs["Q"], np.float32)[0]      # (L, D)
    K = np.asarray(inputs["K"], np.float32)[0]
    V = np.asarray(inputs["V"], np.float32)[0]
    Wq = np.asarray(inputs["Wq"], np.float32)
    Wk = np.asarray(inputs["Wk"], np.float32)
    Wv = np.asarray(inputs["Wv"], np.float32)
    Wo = np.asarray(inputs["Wo"], np.float32)
    bq = np.asarray(inputs["bq"], np.float32)
    bk = np.asarray(inputs["bk"], np.float32)
    E = np.asarray(inputs["rel_emb"], np.float32)   # (4096, 64)
    u_b = np.asarray(inputs["u_bias"], np.float32)  # (16, 64)
    v_b = np.asarray(inputs["v_bias"], np.float32)

    bf = ml_dtypes.bfloat16
    QT = np.ascontiguousarray(Q.T).astype(bf)
    KT = np.ascontiguousarray(K.T).astype(bf)
    VT = np.ascontiguousarray(V.T).astype(bf)
    FT = np.ascontiguousarray(E[::-1].T).astype(bf)  # (64, 4096)

    in_maps = []
    for c in range(NCORES):
        ds = slice(DH * c, DH * c + DH)
        urep = np.concatenate([u_b[2 * c], u_b[2 * c + 1]])[:, None]
        vrep = np.concatenate([v_b[2 * c], v_b[2 * c + 1]])[:, None]
        in_maps.append({
            "qt": QT, "kt": KT, "vt": VT, "ft": FT,
            "wqt": np.ascontiguousarray(Wq[ds].T).astype(bf),
            "wkt": np.ascontiguousarray(Wk[ds].T).astype(bf),
            "wvt": np.ascontiguousarray(Wv[ds].T).astype(bf),
            "wot": np.ascontiguousarray(Wo[:, ds].T).astype(bf),
            "ubias": ((bq[ds, None] + urep) / 8.0).astype(np.float32),
            "vbias": ((bq[ds, None] + vrep) / 8.0).astype(np.float32),
            "kbias": bk[ds, None].astype(np.float32),
            "identb": np.eye(128, dtype=bf),
        })
    return in_maps


def kernel(**inputs) -> np.ndarray:
    Wo = np.asarray(inputs["Wo"], np.float32)
    bo = np.asarray(inputs["bo"], np.float32)
    bv = np.asarray(inputs["bv"], np.float32)

    in_maps = make_in_maps(inputs)
    global _LAST_IN_MAPS
    _LAST_IN_MAPS = in_maps
    nc = _get_module()
    res = run_bass_kernel_spmd(nc, in_maps, core_ids=list(range(NCORES)))
    acc = np.zeros((D, L), np.float64)
    for r in res.results:
        acc += r["opart"].astype(np.float64)
    out = acc.T.astype(np.float32) + bo[None, :] + (bv @ Wo.T)[None, :]
    return out[None, :, :]
